# revision 16
# baseline (speedup 1.0000x reference)
"""Trainium2 Bass kernel for nn_Decoder_481036337511.

Computation: dic = normalized real dictionary [T=1024, 1+4*4096] built from
rr/theta; out = einsum('tk,bkd->btd', dic, x) with x [4, 16385, 2048].

Strategy (8 cores, pure data parallel on D):
  - Each core gets x[:, :, c*256:(c+1)*256] and computes out[:, :, c*256:...].
  - Structure: dic columns = [ones, A, S*A, B, S*B] where A = r^t cos(t th),
    B = r^t sin(t th), S = diag((-1)^t). Column norms of S*A equal those of A,
    so with U=x1+x2, V=x1-x2, W=x3+x4, Z=x3-x4:
       out[even t] = Abar @ U + Bbar @ W + x0/32
       out[odd  t] = Abar @ V + Bbar @ Z + x0/32
    This halves the GEMM FLOPs. Matmuls run in fp16 (1 cycle/row on PE).
  - Dictionary built on-device:
      pw = r^t by repeated doubling on VectorE (no Exp table needed),
      sin/cos via ACT Sin after a fused (t*th mod 2pi) range reduction
      (HW Sin only accepts [-pi, pi]); the sign flip from sin(m - pi) =
      -sin(m) is folded into the multiply scalars.
      B block is pre-scaled by 1/max(r*sin th, 1e-30) before fp16 rounding
      (keeps near-zero-norm columns representable); per-column 1/sqrt(G2)
      via DVE Newton iteration; exact-zero columns masked to 0 to match the
      reference's G==0 semantics.
  - Output per core is [B, 256, 1024] laid out [d, parity-major t]; host
    reassembles to [B, 1024, 2048].
"""

import numpy as np
from contextlib import ExitStack

import concourse.bass as bass
import concourse.bacc as bacc
import concourse.mybir as mybir
from concourse import tile
from concourse import bass_utils

F32 = mybir.dt.float32
F16 = mybir.dt.float16
I32 = mybir.dt.int32
AF = mybir.ActivationFunctionType
OP = mybir.AluOpType

N_CORES = 8
PI = float(np.pi)
TWO_PI = float(2 * np.pi)
RSQRT_MAGIC = 0x5F3759DF
RND_BIG = 12582912.0  # 2^23 + 2^22: (q + BIG) - BIG == round(q) for |q| < 2^22


def build_kernel_nc(B=4, DSH=256, KC=32, T=1024, XG=1, NEWTON=3):
    """Emit the per-core Bass program.

    B: batches; DSH: d columns per core; KC: number of 128-pole chunks;
    T: timesteps; XG: pole-chunks per x-load group.
    """
    NP_ = KC * 128          # poles
    KTOT = 1 + 4 * NP_      # rows of x
    TH = T // 2             # 512 per parity
    DH = DSH // 128         # d-half count per core

    nc = bacc.Bacc("TRN2", target_bir_lowering=False, debug=False)

    rr_d = nc.dram_tensor("rr", [NP_], F32, kind="ExternalInput")
    th_d = nc.dram_tensor("theta", [NP_], F32, kind="ExternalInput")
    x_d = nc.dram_tensor("x", [B, KTOT, DSH], F32, kind="ExternalInput")
    out_d = nc.dram_tensor("out", [B, DSH, T], F32, kind="ExternalOutput")

    with tile.TileContext(nc) as tc, ExitStack() as ctx:
        const = ctx.enter_context(tc.tile_pool(name="const", bufs=1))
        pwp = ctx.enter_context(tc.tile_pool(name="pwp", bufs=2))
        rp = ctx.enter_context(tc.tile_pool(name="rp", bufs=1))
        csp = ctx.enter_context(tc.tile_pool(name="csp", bufs=1))
        stg = ctx.enter_context(tc.tile_pool(name="stg", bufs=2))
        tiny = ctx.enter_context(tc.tile_pool(name="tiny", bufs=4))
        xp = ctx.enter_context(tc.tile_pool(name="xp", bufs=2))
        uvp = ctx.enter_context(tc.tile_pool(name="uv", bufs=2))
        outp = ctx.enter_context(tc.tile_pool(name="outp", bufs=2))
        psp = ctx.enter_context(
            tc.tile_pool(name="ps", bufs=2, space=bass.MemorySpace.PSUM)
        )

        # ---- setup ----------------------------------------------------
        rr_t = const.tile([128, KC], F32, tag="rr")
        th_t = const.tile([128, KC], F32, tag="th")
        nc.sync.dma_start(rr_t[:], rr_d[:].rearrange("(kc p) -> p kc", p=128))
        nc.sync.dma_start(th_t[:], th_d[:].rearrange("(kc p) -> p kc", p=128))

        iota_i = rp.tile([128, TH], I32, tag="q", name="iota_i")
        nc.gpsimd.iota(iota_i[:], pattern=[[1, TH]], base=0,
                       channel_multiplier=0)
        iota_f = const.tile([128, TH], F32, tag="iotaf")
        nc.vector.tensor_copy(iota_f[:], iota_i[:])

        # ones-column bias: x[b,0,d] / sqrt(T), per (dh, b)
        x0s = const.tile([128, DH * B], F32, tag="x0s")
        for b in range(B):
            nc.sync.dma_start(
                x0s[:, b * DH:(b + 1) * DH],
                x_d[b, 0, :].rearrange("(dh p) -> p dh", p=128),
            )
        x0sc = const.tile([128, DH * B], F32, tag="x0sc")
        nc.vector.tensor_scalar_mul(x0sc[:], x0s[:], 1.0 / float(np.sqrt(T)))

        # B-block pre-scale: sB = 1/max(r*sin(th), 1e-11), mask for
        # exact-zero columns.  cos(th)/sin(th) seed the odd-t derivation.
        sinth = const.tile([128, KC], F32, tag="sinth")
        nc.scalar.activation(sinth[:], th_t[:], AF.Sin, bias=0.0, scale=1.0)
        hpi = const.tile([128, 1], F32, tag="hpi")
        nc.vector.memset(hpi[:], PI / 2)
        costh = const.tile([128, KC], F32, tag="costh")
        nc.scalar.activation(costh[:], th_t[:], AF.Sin, bias=hpi[:],
                             scale=1.0)
        rs = const.tile([128, KC], F32, tag="rs")
        nc.vector.tensor_mul(rs[:], sinth[:], rr_t[:])
        rsc = const.tile([128, KC], F32, tag="rsc")
        nc.vector.tensor_scalar_max(rsc[:], rs[:], 1e-11)
        sB = const.tile([128, KC], F32, tag="sB")
        nc.vector.reciprocal(sB[:], rsc[:])
        maskB = const.tile([128, KC], F32, tag="maskB")
        nc.vector.tensor_scalar(maskB[:], rs[:], 0.0, None, op0=OP.is_gt)
        # theta/pi, so that q = j*(th/pi) = (2j)*th/(2pi) counts turns
        thp = const.tile([128, KC], F32, tag="thp")
        nc.vector.tensor_scalar_mul(thp[:], th_t[:], 1.0 / PI)

        # (r^2)^(2^j) per pole (doubling build of r^(2j), parity-major pw)
        r2j = const.tile([128, 10, KC], F32, tag="r2j")
        nc.vector.tensor_mul(r2j[:, 0], rr_t[:], rr_t[:])
        for j in range(1, 10):
            nc.vector.tensor_mul(r2j[:, j], r2j[:, j - 1], r2j[:, j - 1])

        # ---- dictionary -----------------------------------------------
        adict = const.tile([128, KC, 2, TH], F16, tag="adict")
        bdict = const.tile([128, KC, 2, TH], F16, tag="bdict")
        g2 = const.tile([128, 2 * KC], F32, tag="g2")
        invg = const.tile([128, 2 * KC], F32, tag="invg")
        invgm = const.tile([128, KC], F32, tag="invgm")

        for kc in range(KC):
            # pw[par, j] = r^(2j+par), parity-major, built by doubling r^2
            pw = pwp.tile([128, 2, TH], F32, tag="pw")
            nc.vector.memset(pw[:, 0, 0:1], 1.0)
            nc.vector.tensor_copy(pw[:, 0, 1:2], r2j[:, 0, kc:kc + 1])
            n = 2
            j = 1
            while n < TH:
                nc.vector.tensor_scalar(pw[:, 0, n:2 * n], pw[:, 0, 0:n],
                                        r2j[:, j, kc:kc + 1], None,
                                        op0=OP.mult)
                n *= 2
                j += 1
            nc.vector.tensor_scalar(pw[:, 1, :], pw[:, 0, :],
                                    rr_t[:, kc:kc + 1], None, op0=OP.mult)
            # q = (2j)*th/(2pi) in turns; d = q - round(q) in [-1/2, 1/2]
            q = rp.tile([128, TH], F32, tag="q", name="q")
            nc.vector.tensor_scalar(q[:], iota_f[:], thp[:, kc:kc + 1], None,
                                    op0=OP.mult)
            qr = rp.tile([128, TH], F32, tag="qr", name="qr")
            nc.vector.tensor_scalar(qr[:], q[:], RND_BIG, -RND_BIG,
                                    op0=OP.add, op1=OP.add)
            d_t = rp.tile([128, TH], F32, tag="d", name="d_t")
            nc.vector.scalar_tensor_tensor(d_t[:], qr[:], -1.0, q[:],
                                           op0=OP.mult, op1=OP.add)
            dc = rp.tile([128, TH], F32, tag="dc", name="dc")
            nc.vector.tensor_scalar(dc[:], d_t[:], 0.5, -0.5,
                                    op0=OP.min, op1=OP.max)
            # cos stream: d2 = wrap(d + 1/4) to [-1/2, 1/2]
            e_t = rp.tile([128, TH], F32, tag="e", name="e_t")
            nc.vector.tensor_scalar(e_t[:], d_t[:], 0.25, None, op0=OP.add)
            g_t = rp.tile([128, TH], F32, tag="g", name="g_t")
            nc.vector.tensor_scalar(g_t[:], e_t[:], 0.5, None, op0=OP.is_gt)
            d2 = rp.tile([128, TH], F32, tag="d", name="d2")
            nc.vector.tensor_sub(d2[:], e_t[:], g_t[:])
            dc2 = rp.tile([128, TH], F32, tag="dc2", name="dc2")
            nc.vector.tensor_scalar(dc2[:], d2[:], 0.5, -0.5,
                                    op0=OP.min, op1=OP.max)
            # even-t sin/cos via ACT; odd-t by one angle-addition (GPSIMD)
            c_t = csp.tile([128, 2, TH], F32, tag="c", name="c_t")
            s_t = csp.tile([128, 2, TH], F32, tag="s", name="s_t")
            nc.scalar.activation(s_t[:, 0], dc[:], AF.Sin, bias=0.0,
                                 scale=TWO_PI)
            nc.scalar.activation(c_t[:, 0], dc2[:], AF.Sin, bias=0.0,
                                 scale=TWO_PI)
            nc.vector.memset(s_t[:, 0, 0:1], 0.0)
            nc.vector.memset(c_t[:, 0, 0:1], 1.0)
            ta = rp.tile([128, TH], F32, tag="e", name="ta")
            nc.gpsimd.tensor_scalar(ta[:], s_t[:, 0], sinth[:, kc:kc + 1],
                                    None, op0=OP.mult)
            tb = rp.tile([128, TH], F32, tag="g", name="tb")
            nc.gpsimd.tensor_scalar(tb[:], c_t[:, 0], costh[:, kc:kc + 1],
                                    None, op0=OP.mult)
            nc.gpsimd.tensor_sub(c_t[:, 1], tb[:], ta[:])
            tc_ = rp.tile([128, TH], F32, tag="q", name="tc_")
            nc.gpsimd.tensor_scalar(tc_[:], s_t[:, 0], costh[:, kc:kc + 1],
                                    None, op0=OP.mult)
            td = rp.tile([128, TH], F32, tag="qr", name="td")
            nc.gpsimd.tensor_scalar(td[:], c_t[:, 0], sinth[:, kc:kc + 1],
                                    None, op0=OP.mult)
            nc.gpsimd.tensor_add(s_t[:, 1], tc_[:], td[:])
            # unnormalized blocks (fp16), parity-major
            ast = stg.tile([128, 2, TH], F16, tag="ast")
            bst = stg.tile([128, 2, TH], F16, tag="bst")
            nc.vector.tensor_mul(ast[:], pw[:], c_t[:])
            nc.vector.scalar_tensor_tensor(bst[:], s_t[:],
                                           sB[:, kc:kc + 1], pw[:],
                                           op0=OP.mult, op1=OP.mult)
            # column sums of squares (scratch reuses the dead pw slots, f32)
            sq_a = pwp.tile([128, 2, TH], F32, tag="pw", name="sq_a")
            nc.vector.tensor_mul(sq_a[:], ast[:], ast[:])
            nc.vector.tensor_reduce(g2[:, 2 * kc:2 * kc + 1], sq_a[:],
                                    axis=mybir.AxisListType.XY, op=OP.add)
            sq_b = pwp.tile([128, 2, TH], F32, tag="pw", name="sq_b")
            nc.vector.tensor_mul(sq_b[:], bst[:], bst[:])
            nc.vector.tensor_reduce(g2[:, 2 * kc + 1:2 * kc + 2], sq_b[:],
                                    axis=mybir.AxisListType.XY, op=OP.add)
            # invg = 1/sqrt(max(g2, 1e-30)) via Newton
            gc = tiny.tile([128, 2], F32, tag="gc", name="gc")
            nc.vector.tensor_scalar_max(gc[:], g2[:, 2 * kc:2 * kc + 2],
                                        1e-30)
            y0i = tiny.tile([128, 2], I32, tag="y0i", name="y0i")
            nc.vector.tensor_scalar(y0i[:], gc[:].bitcast(I32), 1, None,
                                    op0=OP.arith_shift_right)
            y_t = tiny.tile([128, 2], F32, tag="y", name="y_t")
            nc.vector.tensor_scalar(y_t[:].bitcast(I32), y0i[:], -1,
                                    RSQRT_MAGIC, op0=OP.mult, op1=OP.add)
            for it in range(NEWTON):
                yy = tiny.tile([128, 2], F32, tag="yy", name="yy")
                nc.vector.tensor_mul(yy[:], y_t[:], y_t[:])
                ee = tiny.tile([128, 2], F32, tag="ee", name="ee")
                nc.vector.tensor_mul(ee[:], yy[:], gc[:])
                ff = tiny.tile([128, 2], F32, tag="ff", name="ff")
                nc.vector.tensor_scalar(ff[:], ee[:], -0.5, 1.5,
                                        op0=OP.mult, op1=OP.add)
                yn = tiny.tile([128, 2], F32, tag="y", name="yn")
                nc.vector.tensor_mul(yn[:], y_t[:], ff[:])
                y_t = yn
            nc.vector.tensor_copy(invg[:, 2 * kc:2 * kc + 2], y_t[:])
            nc.vector.tensor_mul(invgm[:, kc:kc + 1],
                                 invg[:, 2 * kc + 1:2 * kc + 2],
                                 maskB[:, kc:kc + 1])
            # normalize into parity-major dict (ACT Copy with per-part scale)
            nc.scalar.activation(adict[:, kc], ast[:], AF.Copy,
                                 bias=0.0, scale=invg[:, 2 * kc:2 * kc + 1])
            nc.scalar.activation(bdict[:, kc], bst[:], AF.Copy,
                                 bias=0.0, scale=invgm[:, kc:kc + 1])

        # ---- GEMM over batches ---------------------------------------
        for b in range(B):
            ps_e = [psp.tile([128, TH], F32, tag=f"pe{dh}", name=f"pse{dh}")
                    for dh in range(DH)]
            ps_o = [psp.tile([128, TH], F32, tag=f"po{dh}", name=f"pso{dh}")
                    for dh in range(DH)]
            for g in range(KC // XG):
                xt = []
                for j in range(4):
                    t = xp.tile([128, XG, DSH], F32, tag=f"x{j}",
                                name=f"xt{j}")
                    r0 = 1 + j * NP_ + g * XG * 128
                    nc.sync.dma_start(
                        t[:],
                        x_d[b, r0:r0 + XG * 128, :].rearrange(
                            "(kc p) d -> p kc d", p=128),
                    )
                    xt.append(t)
                u_t = uvp.tile([128, XG, DSH], F16, tag="u", name="u_t")
                v_t = uvp.tile([128, XG, DSH], F16, tag="v", name="v_t")
                w_t = uvp.tile([128, XG, DSH], F16, tag="w", name="w_t")
                z_t = uvp.tile([128, XG, DSH], F16, tag="z", name="z_t")
                nc.vector.tensor_add(u_t[:], xt[0][:], xt[1][:])
                nc.vector.tensor_sub(v_t[:], xt[0][:], xt[1][:])
                nc.gpsimd.tensor_add(w_t[:], xt[2][:], xt[3][:])
                nc.gpsimd.tensor_sub(z_t[:], xt[2][:], xt[3][:])
                for i in range(XG):
                    kc = g * XG + i
                    first = kc == 0
                    last = kc == KC - 1
                    for dh in range(DH):
                        dsl = slice(dh * 128, (dh + 1) * 128)
                        nc.tensor.matmul(ps_e[dh][:], u_t[:, i, dsl],
                                         adict[:, kc, 0, :],
                                         start=first, stop=False)
                        nc.tensor.matmul(ps_o[dh][:], v_t[:, i, dsl],
                                         adict[:, kc, 1, :],
                                         start=first, stop=False)
                        nc.tensor.matmul(ps_e[dh][:], w_t[:, i, dsl],
                                         bdict[:, kc, 0, :],
                                         start=False, stop=last)
                        nc.tensor.matmul(ps_o[dh][:], z_t[:, i, dsl],
                                         bdict[:, kc, 1, :],
                                         start=False, stop=last)
            for dh in range(DH):
                col = b * DH + dh
                ob_e = outp.tile([128, TH], F32, tag="ob", name="ob_e")
                ob_o = outp.tile([128, TH], F32, tag="ob", name="ob_o")
                nc.scalar.activation(ob_e[:], ps_e[dh][:], AF.Identity,
                                     bias=x0sc[:, col:col + 1], scale=1.0)
                nc.scalar.activation(ob_o[:], ps_o[dh][:], AF.Identity,
                                     bias=x0sc[:, col:col + 1], scale=1.0)
                rows = slice(dh * 128, (dh + 1) * 128)
                nc.sync.dma_start(out_d[b, rows, 0:TH], ob_e[:])
                nc.sync.dma_start(out_d[b, rows, TH:T], ob_o[:])
    nc.compile()
    return nc


_NC_CACHE = {}


def _get_nc(key, **kw):
    if key not in _NC_CACHE:
        _NC_CACHE[key] = build_kernel_nc(**kw)
    return _NC_CACHE[key]


def assemble_output(core_outs, B=4, T=1024, D=2048):
    """core_outs: list of [B, DSH, T] arrays (parity-major t) -> [B, T, D]."""
    dsh = D // len(core_outs)
    th = T // 2
    out = np.empty((B, T, D), dtype=np.float32)
    for c, oc in enumerate(core_outs):
        dsl = slice(c * dsh, (c + 1) * dsh)
        out[:, 0::2, dsl] = np.swapaxes(oc[:, :, :th], 1, 2)
        out[:, 1::2, dsl] = np.swapaxes(oc[:, :, th:], 1, 2)
    return out


def kernel(rr, theta, x, trace=False, trace_kwargs=None):
    rr = np.ascontiguousarray(np.asarray(rr, dtype=np.float32))
    theta = np.ascontiguousarray(np.asarray(theta, dtype=np.float32))
    x = np.asarray(x, dtype=np.float32)
    B, KTOT, D = x.shape
    dsh = D // N_CORES
    nc = _get_nc("full")
    in_maps = []
    for c in range(N_CORES):
        in_maps.append({
            "rr": rr,
            "theta": theta,
            "x": np.ascontiguousarray(x[:, :, c * dsh:(c + 1) * dsh]),
        })
    kw = {}
    if trace:
        kw = {"trace": True, "trace_kwargs": trace_kwargs or {}}
    res = bass_utils.run_bass_kernel_spmd(nc, in_maps,
                                          core_ids=list(range(N_CORES)), **kw)
    out = assemble_output([res.results[c]["out"] for c in range(N_CORES)],
                          B=B, T=1024, D=D)
    if trace:
        return out, res
    return out


# revision 29
# speedup vs baseline: 465.1689x; 465.1689x over previous
"""Trainium2 Bass kernel for nn_Decoder_481036337511.

Computation: dic = normalized real dictionary [T=1024, 1+4*4096] built from
rr/theta; out = einsum('tk,bkd->btd', dic, x) with x [4, 16385, 2048].

Strategy (8 cores, pure data parallel on D):
  - Each core gets x[:, :, c*256:(c+1)*256] and computes out[:, :, c*256:...].
  - Structure: dic columns = [ones, A, S*A, B, S*B] where A = r^t cos(t th),
    B = r^t sin(t th), S = diag((-1)^t). Column norms of S*A equal those of A,
    so with U=x1+x2, V=x1-x2, W=x3+x4, Z=x3-x4:
       out[even t] = Abar @ U + Bbar @ W + x0/32
       out[odd  t] = Abar @ V + Bbar @ Z + x0/32
    This halves the GEMM FLOPs. Matmuls run in fp16 (1 cycle/row on PE).
  - Dictionary built on-device:
      pw = r^t by repeated doubling on VectorE (no Exp table needed),
      sin/cos via ACT Sin after a fused (t*th mod 2pi) range reduction
      (HW Sin only accepts [-pi, pi]); the sign flip from sin(m - pi) =
      -sin(m) is folded into the multiply scalars.
      B block is pre-scaled by 1/max(r*sin th, 1e-30) before fp16 rounding
      (keeps near-zero-norm columns representable); per-column 1/sqrt(G2)
      via DVE Newton iteration; exact-zero columns masked to 0 to match the
      reference's G==0 semantics.
  - Output per core is [B, 256, 1024] laid out [d, parity-major t]; host
    reassembles to [B, 1024, 2048].
"""

import numpy as np
from contextlib import ExitStack

import concourse.bass as bass
import concourse.bacc as bacc
import concourse.mybir as mybir
from concourse import tile
from concourse import bass_utils

F32 = mybir.dt.float32
F16 = mybir.dt.float16
I32 = mybir.dt.int32
AF = mybir.ActivationFunctionType
OP = mybir.AluOpType

N_CORES = 8
PI = float(np.pi)
TWO_PI = float(2 * np.pi)
RSQRT_MAGIC = 0x5F3759DF
RND_BIG = 12582912.0  # 2^23 + 2^22: (q + BIG) - BIG == round(q) for |q| < 2^22


def build_kernel_nc(B=4, DSH=256, KC=32, T=1024, XG=2, NEWTON=2):
    """Emit the per-core Bass program.

    B: batches; DSH: d columns per core; KC: number of 128-pole chunks;
    T: timesteps; XG: pole-chunks per x-load group.
    """
    NP_ = KC * 128          # poles
    KTOT = 1 + 4 * NP_      # rows of x
    TH = T // 2             # 512 per parity
    DH = DSH // 128         # d-half count per core

    nc = bacc.Bacc("TRN2", target_bir_lowering=False, debug=False)

    rr_d = nc.dram_tensor("rr", [NP_], F32, kind="ExternalInput")
    th_d = nc.dram_tensor("theta", [NP_], F32, kind="ExternalInput")
    x_d = nc.dram_tensor("x", [B, KTOT, DSH], F32, kind="ExternalInput")
    out_d = nc.dram_tensor("out", [B, DSH, T], F32, kind="ExternalOutput")

    with tile.TileContext(nc) as tc, ExitStack() as ctx:
        const = ctx.enter_context(tc.tile_pool(name="const", bufs=1))
        pwp = ctx.enter_context(tc.tile_pool(name="pwp", bufs=2))
        sqp = ctx.enter_context(tc.tile_pool(name="sqp", bufs=2))
        rp = ctx.enter_context(tc.tile_pool(name="rp", bufs=2))
        csp = ctx.enter_context(tc.tile_pool(name="csp", bufs=1))
        stg = ctx.enter_context(tc.tile_pool(name="stg", bufs=2))
        tiny = ctx.enter_context(tc.tile_pool(name="tiny", bufs=4))
        xp = ctx.enter_context(tc.tile_pool(name="xp", bufs=2))
        uvp = ctx.enter_context(tc.tile_pool(name="uv", bufs=2))
        outp = ctx.enter_context(tc.tile_pool(name="outp", bufs=1))
        psp = ctx.enter_context(
            tc.tile_pool(name="ps", bufs=2, space=bass.MemorySpace.PSUM)
        )

        # ---- setup ----------------------------------------------------
        rr_t = const.tile([128, KC], F32, tag="rr")
        th_t = const.tile([128, KC], F32, tag="th")
        nc.sync.dma_start(rr_t[:], rr_d[:].rearrange("(kc p) -> p kc", p=128))
        nc.sync.dma_start(th_t[:], th_d[:].rearrange("(kc p) -> p kc", p=128))

        iota_i = rp.tile([128, TH], I32, tag="q", name="iota_i")
        nc.gpsimd.iota(iota_i[:], pattern=[[1, TH]], base=0,
                       channel_multiplier=0)
        iota_f = const.tile([128, TH], F32, tag="iotaf")
        nc.vector.tensor_copy(iota_f[:], iota_i[:])

        # ones-column bias: x[b,0,d] / sqrt(T), per (dh, b)
        x0s = const.tile([128, DH * B], F32, tag="x0s")
        for b in range(B):
            nc.sync.dma_start(
                x0s[:, b * DH:(b + 1) * DH],
                x_d[b, 0, :].rearrange("(dh p) -> p dh", p=128),
            )
        x0sc = const.tile([128, DH * B], F32, tag="x0sc")
        nc.vector.tensor_scalar_mul(x0sc[:], x0s[:], 1.0 / float(np.sqrt(T)))

        # Closed-form column norms.  With R = r^2, z = R e^{2i th},
        # S0 = sum_t R^t and C = sum_t z^t (t = 0..T-1, geometric series):
        #   G_A^2 = (S0 + Re C)/2,   G_B^2 = (S0 - Re C)/2.
        # Computed cancellation-free: 1-R = (1-r)(1+r), Re(1-z) =
        # (1-R) + 2R sin^2(th), and the difference S0 - Re C is evaluated
        # as Re[N/D] with N = (R-z) - R^T(1-z) + z^T(1-R), D = (1-R)(1-z),
        # whose leading term 2R sin^2(th) is a positive product (no
        # subtractive cancellation for near-zero-norm columns).
        sinth = const.tile([128, KC], F32, tag="sinth")
        nc.scalar.activation(sinth[:], th_t[:], AF.Sin, bias=0.0, scale=1.0)
        hpi = const.tile([128, 1], F32, tag="hpi")
        nc.vector.memset(hpi[:], PI / 2)
        bigp = const.tile([128, 1], F32, tag="bigp")
        nc.vector.memset(bigp[:], RND_BIG)
        bigm = const.tile([128, 1], F32, tag="bigm")
        nc.vector.memset(bigm[:], -RND_BIG)
        costh = const.tile([128, KC], F32, tag="costh")
        nc.scalar.activation(costh[:], th_t[:], AF.Sin, bias=hpi[:],
                             scale=1.0)
        rs = const.tile([128, KC], F32, tag="rs")
        nc.vector.tensor_mul(rs[:], sinth[:], rr_t[:])
        maskB = const.tile([128, KC], F32, tag="maskB")
        nc.vector.tensor_scalar(maskB[:], rs[:], 0.0, None, op0=OP.is_gt)
        # theta/pi, so that q = j*(th/pi) = (2j)*th/(2pi) counts turns
        thp = const.tile([128, KC], F32, tag="thp")
        nc.vector.tensor_scalar_mul(thp[:], th_t[:], 1.0 / PI)

        # (r^2)^(2^j) per pole (doubling build of r^(2j), parity-major pw)
        r2j = const.tile([128, 10, KC], F32, tag="r2j")
        nc.vector.tensor_mul(r2j[:, 0], rr_t[:], rr_t[:])
        for j in range(1, 10):
            nc.vector.tensor_mul(r2j[:, j], r2j[:, j - 1], r2j[:, j - 1])

        cfp = ctx.enter_context(tc.tile_pool(name="cfp", bufs=1))

        def cf(name):
            return cfp.tile([128, KC], F32, tag=name, name=name)

        R_ = r2j[:, 0]
        rt = cf("rt")                      # R^T = r^2048
        nc.vector.tensor_mul(rt[:], r2j[:, 9], r2j[:, 9])
        omr = cf("omr")
        nc.vector.tensor_scalar(omr[:], rr_t[:], -1.0, 1.0,
                                op0=OP.mult, op1=OP.add)
        opr = cf("opr")
        nc.vector.tensor_scalar(opr[:], rr_t[:], 1.0, None, op0=OP.add)
        omR = cf("omR")
        nc.vector.tensor_mul(omR[:], omr[:], opr[:])
        ssq2 = cf("ssq2")                  # 2 sin^2(th)
        nc.vector.scalar_tensor_tensor(ssq2[:], sinth[:], 2.0, sinth[:],
                                       op0=OP.mult, op1=OP.mult)
        s2t = cf("s2t")                    # sin(2 th)
        nc.vector.scalar_tensor_tensor(s2t[:], sinth[:], 2.0, costh[:],
                                       op0=OP.mult, op1=OP.mult)
        zim = cf("zim")                    # Im z = R sin(2 th)
        nc.vector.tensor_mul(zim[:], R_, s2t[:])
        rmz = cf("rmz")                    # Re(R - z) = 2 R sin^2(th)
        nc.vector.tensor_mul(rmz[:], R_, ssq2[:])
        a1r = cf("a1r")                    # Re(1 - z)
        nc.vector.tensor_add(a1r[:], omR[:], rmz[:])
        # angle 2*T*th mod 2pi via the turn trick (T = 1024)
        qq = cf("qq")
        nc.vector.tensor_scalar(qq[:], thp[:], 1024.0, None, op0=OP.mult)
        qqr = cf("qqr")
        nc.vector.tensor_scalar(qqr[:], qq[:], RND_BIG, -RND_BIG,
                                op0=OP.add, op1=OP.add)
        dd = cf("dd")
        nc.vector.scalar_tensor_tensor(dd[:], qqr[:], -1.0, qq[:],
                                       op0=OP.mult, op1=OP.add)
        ddc = cf("ddc")
        nc.vector.tensor_scalar(ddc[:], dd[:], 0.5, -0.5,
                                op0=OP.min, op1=OP.max)
        adt = cf("adt")
        nc.vector.tensor_scalar(adt[:].bitcast(I32), dd[:].bitcast(I32),
                                0x7FFFFFFF, None, op0=OP.bitwise_and)
        sT = cf("sT")
        nc.scalar.activation(sT[:], ddc[:], AF.Sin, bias=0.0, scale=TWO_PI)
        cT = cf("cT")
        nc.scalar.activation(cT[:], adt[:], AF.Sin, bias=hpi[:],
                             scale=-TWO_PI)
        zTr = cf("zTr")
        nc.vector.tensor_mul(zTr[:], rt[:], cT[:])
        zTi = cf("zTi")
        nc.vector.tensor_mul(zTi[:], rt[:], sT[:])
        omrt = cf("omrt")                  # 1 - R^T
        nc.vector.tensor_scalar(omrt[:], rt[:], -1.0, 1.0,
                                op0=OP.mult, op1=OP.add)
        rrec = cf("rrec")
        nc.vector.reciprocal(rrec[:], omR[:])
        s0_ = cf("s0_")                    # S0 = (1-R^T)/(1-R)
        nc.vector.tensor_mul(s0_[:], omrt[:], rrec[:])
        # Re C = ((1-zTr) a1r + zTi zim) / (a1r^2 + zim^2)
        xx = cf("xx")
        nc.vector.tensor_scalar(xx[:], zTr[:], -1.0, 1.0,
                                op0=OP.mult, op1=OP.add)
        n1 = cf("n1")
        nc.vector.tensor_mul(n1[:], xx[:], a1r[:])
        n2 = cf("n2")
        nc.vector.tensor_mul(n2[:], zTi[:], zim[:])
        num = cf("num")
        nc.vector.tensor_add(num[:], n1[:], n2[:])
        dn1 = cf("dn1")
        nc.vector.tensor_mul(dn1[:], a1r[:], a1r[:])
        dn2 = cf("dn2")
        nc.vector.tensor_mul(dn2[:], zim[:], zim[:])
        den = cf("den")
        nc.vector.tensor_add(den[:], dn1[:], dn2[:])
        rden = cf("rden")
        nc.vector.reciprocal(rden[:], den[:])
        reC = cf("reC")
        nc.vector.tensor_mul(reC[:], num[:], rden[:])
        # pack G^2: [:,0,:] = A block, [:,1,:] = B block
        g2t = const.tile([128, 2, KC], F32, tag="g2t")
        nc.vector.tensor_add(g2t[:, 0], s0_[:], reC[:])
        nc.vector.tensor_scalar_mul(g2t[:, 0], g2t[:, 0], 0.5)
        # G_B^2 = Re[N/D]/2
        nr1 = cf("nr1")
        nc.vector.tensor_mul(nr1[:], rt[:], a1r[:])
        nr2 = cf("nr2")
        nc.vector.tensor_mul(nr2[:], zTr[:], omR[:])
        nre = cf("nre")
        nc.vector.tensor_sub(nre[:], rmz[:], nr1[:])
        nc.vector.tensor_add(nre[:], nre[:], nr2[:])
        ni1 = cf("ni1")
        nc.vector.tensor_mul(ni1[:], zim[:], omrt[:])
        ni2 = cf("ni2")
        nc.vector.tensor_mul(ni2[:], zTi[:], omR[:])
        nim = cf("nim")
        nc.vector.tensor_sub(nim[:], ni2[:], ni1[:])
        dre = cf("dre")
        nc.vector.tensor_mul(dre[:], omR[:], a1r[:])
        dimp = cf("dimp")                  # -Im D
        nc.vector.tensor_mul(dimp[:], omR[:], zim[:])
        m1_ = cf("m1_")
        nc.vector.tensor_mul(m1_[:], nre[:], dre[:])
        m2_ = cf("m2_")
        nc.vector.tensor_mul(m2_[:], nim[:], dimp[:])
        mnum = cf("mnum")
        nc.vector.tensor_sub(mnum[:], m1_[:], m2_[:])
        e1_ = cf("e1_")
        nc.vector.tensor_mul(e1_[:], dre[:], dre[:])
        e2_ = cf("e2_")
        nc.vector.tensor_mul(e2_[:], dimp[:], dimp[:])
        eden = cf("eden")
        nc.vector.tensor_add(eden[:], e1_[:], e2_[:])
        rede = cf("rede")
        nc.vector.reciprocal(rede[:], eden[:])
        nc.vector.tensor_mul(g2t[:, 1], mnum[:], rede[:])
        nc.vector.tensor_scalar_mul(g2t[:, 1], g2t[:, 1], 0.5)
        # invg = 1/sqrt(max(g2, 1e-30)) via Newton, one pass over both blocks
        gcl = const.tile([128, 2, KC], F32, tag="gcl")
        nc.vector.tensor_scalar_max(gcl[:], g2t[:], 1e-30)
        y0i = const.tile([128, 2, KC], I32, tag="y0i")
        nc.vector.tensor_scalar(y0i[:], gcl[:].bitcast(I32), 1, None,
                                op0=OP.arith_shift_right)
        invgt = const.tile([128, 2, KC], F32, tag="invgt")
        y_t = invgt
        nc.vector.tensor_scalar(y_t[:].bitcast(I32), y0i[:], -1,
                                RSQRT_MAGIC, op0=OP.mult, op1=OP.add)
        yy = const.tile([128, 2, KC], F32, tag="yy")
        ff = const.tile([128, 2, KC], F32, tag="ff")
        for it in range(NEWTON + 1):
            nc.vector.tensor_mul(yy[:], y_t[:], y_t[:])
            nc.vector.tensor_mul(yy[:], yy[:], gcl[:])
            nc.vector.tensor_scalar(ff[:], yy[:], -0.5, 1.5,
                                    op0=OP.mult, op1=OP.add)
            nc.vector.tensor_mul(y_t[:], y_t[:], ff[:])
        invgbm = const.tile([128, KC], F32, tag="invgbm")
        nc.vector.tensor_mul(invgbm[:], invgt[:, 1], maskB[:])

        # ---- dictionary -----------------------------------------------
        adict = const.tile([128, KC, 2, TH], F16, tag="adict")
        bdict = const.tile([128, KC, 2, TH], F16, tag="bdict")

        # Software-pipelined build: emit S0(k), S1(k-1), S2(k-2)
        # interleaved so every engine always has ready work (engines issue
        # in program order with a 4-deep wait queue; a naive per-kc chain
        # crosses engines ~10 times and serializes).
        st = {}

        def s0(kc):
            # pw[par, j] = r^(2j+par) by doubling r^2; q = j*th/pi (turns)
            pw = pwp.tile([128, 2, TH], F32, tag="pw", name="pw")
            nc.vector.memset(pw[:, 0, 0:1], 1.0)
            nc.vector.tensor_copy(pw[:, 0, 1:2], r2j[:, 0, kc:kc + 1])
            n = 2
            j = 1
            while n < TH:
                nc.vector.tensor_scalar(pw[:, 0, n:2 * n], pw[:, 0, 0:n],
                                        r2j[:, j, kc:kc + 1], None,
                                        op0=OP.mult)
                n *= 2
                j += 1
            nc.vector.tensor_scalar(pw[:, 1, :], pw[:, 0, :],
                                    rr_t[:, kc:kc + 1], None, op0=OP.mult)
            q = rp.tile([128, TH], F32, tag="q", name="q")
            nc.scalar.activation(q[:], iota_f[:], AF.Identity, bias=0.0,
                                 scale=thp[:, kc:kc + 1])
            qh = rp.tile([128, TH], F32, tag="qh", name="qh")
            nc.scalar.activation(qh[:], q[:], AF.Identity, bias=bigp[:],
                                 scale=1.0)
            qr = rp.tile([128, TH], F32, tag="qr", name="qr")
            nc.scalar.activation(qr[:], qh[:], AF.Identity, bias=bigm[:],
                                 scale=1.0)
            st[kc] = {"pw": pw, "q": q, "qr": qr}

        def s1(kc):
            z = st[kc]
            d_t = rp.tile([128, TH], F32, tag="d", name="d_t")
            nc.vector.scalar_tensor_tensor(d_t[:], z["qr"][:], -1.0,
                                           z["q"][:], op0=OP.mult,
                                           op1=OP.add)
            dc = rp.tile([128, TH], F32, tag="dc", name="dc")
            nc.vector.tensor_scalar(dc[:], d_t[:], 0.5, -0.5,
                                    op0=OP.min, op1=OP.max)
            # cos(2*pi*d) = sin(pi/2 - 2*pi*|d|), |d| <= 1/2 stays in range
            absd = rp.tile([128, TH], F32, tag="d", name="absd")
            nc.vector.tensor_scalar(absd[:].bitcast(I32),
                                    d_t[:].bitcast(I32), 0x7FFFFFFF, None,
                                    op0=OP.bitwise_and)
            c_t = csp.tile([128, 2, TH], F32, tag="c", name="c_t")
            s_t = csp.tile([128, 2, TH], F32, tag="s", name="s_t")
            nc.scalar.activation(s_t[:, 0], dc[:], AF.Sin, bias=0.0,
                                 scale=TWO_PI)
            nc.scalar.activation(c_t[:, 0], absd[:], AF.Sin, bias=hpi[:],
                                 scale=-TWO_PI)
            nc.vector.memset(s_t[:, 0, 0:1], 0.0)
            nc.vector.memset(c_t[:, 0, 0:1], 1.0)
            z.update(c=c_t, s=s_t)

        def s2(kc):
            z = st[kc]
            c_t, s_t, pw = z["c"], z["s"], z["pw"]
            ta = rp.tile([128, TH], F32, tag="q", name="ta")
            nc.vector.tensor_scalar(ta[:], s_t[:, 0], sinth[:, kc:kc + 1],
                                    None, op0=OP.mult)
            nc.vector.scalar_tensor_tensor(c_t[:, 1], c_t[:, 0],
                                           costh[:, kc:kc + 1], ta[:],
                                           op0=OP.mult, op1=OP.subtract)
            tb = rp.tile([128, TH], F32, tag="qr", name="tb")
            nc.vector.tensor_scalar(tb[:], c_t[:, 0], sinth[:, kc:kc + 1],
                                    None, op0=OP.mult)
            nc.vector.scalar_tensor_tensor(s_t[:, 1], s_t[:, 0],
                                           costh[:, kc:kc + 1], tb[:],
                                           op0=OP.mult, op1=OP.add)
            # normalized fp16 dictionary, written directly
            nc.vector.scalar_tensor_tensor(adict[:, kc], c_t[:],
                                           invgt[:, 0, kc:kc + 1], pw[:],
                                           op0=OP.mult, op1=OP.mult)
            nc.vector.scalar_tensor_tensor(bdict[:, kc], s_t[:],
                                           invgbm[:, kc:kc + 1], pw[:],
                                           op0=OP.mult, op1=OP.mult)
            del st[kc]

        # ---- GEMM --------------------------------------------------
        # b0/b1 are interleaved with the dictionary build (their matmuls
        # consume dict chunks as they land; combines run on Pool which is
        # idle during the dict phase). b2/b3 run after, combines on DVE
        # (idle once the dict is done).
        ps = {}

        def gemm_open(b):
            ps[b] = ([psp.tile([128, TH], F32, tag=f"pe{dh}",
                               name=f"pse{dh}") for dh in range(DH)],
                     [psp.tile([128, TH], F32, tag=f"po{dh}",
                               name=f"pso{dh}") for dh in range(DH)])

        def gemm_load(b, g, eng):
            xt = xp.tile([128, 4, XG, DSH], F32, tag="x", name="xt")
            for i in range(XG):
                nc.sync.dma_start(
                    xt[:, :, i],
                    x_d[b, 1:, :].rearrange(
                        "(blk kc p) d -> p blk kc d", blk=4,
                        kc=KC, p=128)[:, :, g * XG + i],
                )
            u_t = uvp.tile([128, XG, DSH], F16, tag="u", name="u_t")
            v_t = uvp.tile([128, XG, DSH], F16, tag="v", name="v_t")
            w_t = uvp.tile([128, XG, DSH], F16, tag="w", name="w_t")
            z_t = uvp.tile([128, XG, DSH], F16, tag="z", name="z_t")
            e0 = nc.gpsimd if eng == "pool" else nc.vector
            e1 = nc.gpsimd if eng == "pool" else nc.vector
            e0.tensor_add(u_t[:], xt[:, 0], xt[:, 1])
            e0.tensor_sub(v_t[:], xt[:, 0], xt[:, 1])
            e1.tensor_add(w_t[:], xt[:, 2], xt[:, 3])
            e1.tensor_sub(z_t[:], xt[:, 2], xt[:, 3])
            return u_t, v_t, w_t, z_t

        def gemm_kc(b, kc, uvwz):
            u_t, v_t, w_t, z_t = uvwz
            ps_e, ps_o = ps[b]
            i = kc % XG
            first = kc == 0
            last = kc == KC - 1
            for dh in range(DH):
                dsl = slice(dh * 128, (dh + 1) * 128)
                nc.tensor.matmul(ps_e[dh][:], u_t[:, i, dsl],
                                 adict[:, kc, 0, :], start=first,
                                 stop=False)
                nc.tensor.matmul(ps_o[dh][:], v_t[:, i, dsl],
                                 adict[:, kc, 1, :], start=first,
                                 stop=False)
                nc.tensor.matmul(ps_e[dh][:], w_t[:, i, dsl],
                                 bdict[:, kc, 0, :], start=False, stop=last)
                nc.tensor.matmul(ps_o[dh][:], z_t[:, i, dsl],
                                 bdict[:, kc, 1, :], start=False, stop=last)

        def gemm_close(b):
            ps_e, ps_o = ps.pop(b)
            for dh in range(DH):
                col = b * DH + dh
                ob_e = outp.tile([128, TH], F32, tag="ob", name="ob_e")
                ob_o = outp.tile([128, TH], F32, tag="ob", name="ob_o")
                nc.scalar.activation(ob_e[:], ps_e[dh][:], AF.Identity,
                                     bias=x0sc[:, col:col + 1], scale=1.0)
                nc.scalar.activation(ob_o[:], ps_o[dh][:], AF.Identity,
                                     bias=x0sc[:, col:col + 1], scale=1.0)
                rows = slice(dh * 128, (dh + 1) * 128)
                nc.sync.dma_start(out_d[b, rows, 0:TH], ob_e[:])
                nc.sync.dma_start(out_d[b, rows, TH:T], ob_o[:])

        # phase 1: dict stages interleaved with b0/b1 consumption
        p1 = [b for b in (0, 1) if b < B]
        gemm_open(0)
        if 1 in p1:
            gemm_open(1)
        uvwz01 = {}
        for k in range(KC + 2):
            if k < KC:
                s0(k)
            if 1 <= k < KC + 1:
                s1(k - 1)
            if k >= 2:
                kc = k - 2
                s2(kc)
                if kc % XG == 0:
                    g = kc // XG
                    for b in p1:
                        uvwz01[b] = gemm_load(b, g, "pool")
                for b in p1:
                    gemm_kc(b, kc, uvwz01[b])
        for b in p1:
            gemm_close(b)

        # phase 2: remaining batches at full speed, combines on DVE
        for b in range(2, B):
            gemm_open(b)
            for g in range(KC // XG):
                uvwz = gemm_load(b, g, "dve")
                for i in range(XG):
                    gemm_kc(b, g * XG + i, uvwz)
            gemm_close(b)
    nc.compile()
    return nc


_NC_CACHE = {}


def _get_nc(key, **kw):
    if key not in _NC_CACHE:
        _NC_CACHE[key] = build_kernel_nc(**kw)
    return _NC_CACHE[key]


def assemble_output(core_outs, B=4, T=1024, D=2048):
    """core_outs: list of [B, DSH, T] arrays (parity-major t) -> [B, T, D]."""
    dsh = D // len(core_outs)
    th = T // 2
    out = np.empty((B, T, D), dtype=np.float32)
    for c, oc in enumerate(core_outs):
        dsl = slice(c * dsh, (c + 1) * dsh)
        out[:, 0::2, dsl] = np.swapaxes(oc[:, :, :th], 1, 2)
        out[:, 1::2, dsl] = np.swapaxes(oc[:, :, th:], 1, 2)
    return out


def kernel(rr, theta, x, trace=False, trace_kwargs=None):
    rr = np.ascontiguousarray(np.asarray(rr, dtype=np.float32))
    theta = np.ascontiguousarray(np.asarray(theta, dtype=np.float32))
    x = np.asarray(x, dtype=np.float32)
    B, KTOT, D = x.shape
    dsh = D // N_CORES
    nc = _get_nc("full")
    in_maps = []
    for c in range(N_CORES):
        in_maps.append({
            "rr": rr,
            "theta": theta,
            "x": np.ascontiguousarray(x[:, :, c * dsh:(c + 1) * dsh]),
        })
    kw = {}
    if trace:
        kw = {"trace": True, "trace_kwargs": trace_kwargs or {}}
    res = bass_utils.run_bass_kernel_spmd(nc, in_maps,
                                          core_ids=list(range(N_CORES)), **kw)
    out = assemble_output([res.results[c]["out"] for c in range(N_CORES)],
                          B=B, T=1024, D=D)
    if trace:
        return out, res
    return out


# revision 32
# speedup vs baseline: 888.2861x; 1.9096x over previous
"""Trainium2 Bass kernel for nn_Decoder_481036337511.

Computation: dic = normalized real dictionary [T=1024, 1+4*4096] built from
rr/theta; out = einsum('tk,bkd->btd', dic, x) with x [4, 16385, 2048].

Strategy (8 cores, pure data parallel on D):
  - Each core gets x[:, :, c*256:(c+1)*256] and computes out[:, :, c*256:...].
  - Structure: dic columns = [ones, A, S*A, B, S*B] where A = r^t cos(t th),
    B = r^t sin(t th), S = diag((-1)^t). Column norms of S*A equal those of A,
    so with U=x1+x2, V=x1-x2, W=x3+x4, Z=x3-x4:
       out[even t] = Abar @ U + Bbar @ W + x0/sqrt(T)
       out[odd  t] = Abar @ V + Bbar @ Z + x0/sqrt(T)
    This halves the GEMM FLOPs. Matmuls run in fp16 (1 cycle/row on PE),
    stationary = x-side [k,128d] halves (LDWEIGHTS hides under N=512 moving).
  - Dictionary built on-device, software-pipelined across pole chunks:
      pw = r^t by repeated doubling of r^2 (parity-major, no Exp table),
      angles reduced in "turns" q = t*th/2pi with the fp32 big-constant
      round trick; sin from ACT Sin(2pi*d), cos = sin(pi/2 - 2pi*|d|)
      (HW Sin only accepts [-pi, pi]); odd-t sin/cos by one angle addition.
      Column norms via closed-form geometric series (cancellation-free),
      1/sqrt via DVE Newton; normalization folded into the final fp16
      multiply that writes the dict; exact-zero columns masked to match the
      reference's G==0 semantics.
  - b0/b1 matmuls are interleaved with the dict build (combines on Pool);
    b2/b3 run after (combines on DVE). One DMA per (b, chunk) loads all 4
    x-blocks; psum holds 2 batches x 2 d-halves x 2 parities.
  - Output per core is [B, 256, 1024] laid out [d, parity-major t]; host
    reassembles to [B, 1024, 2048].
"""

import numpy as np
from contextlib import ExitStack

import concourse.bass as bass
import concourse.bacc as bacc
import concourse.mybir as mybir
from concourse import tile
from concourse import bass_utils

F32 = mybir.dt.float32
F16 = mybir.dt.float16
I32 = mybir.dt.int32
AF = mybir.ActivationFunctionType
OP = mybir.AluOpType

N_CORES = 8
PI = float(np.pi)
TWO_PI = float(2 * np.pi)
RSQRT_MAGIC = 0x5F3759DF
RND_BIG = 12582912.0  # 2^23 + 2^22: (q + BIG) - BIG == round(q) for |q| < 2^22


def build_kernel_nc(B=4, DSH=256, KC=32, T=1024, XG=2, NEWTON=2):
    """Emit the per-core Bass program.

    B: batches; DSH: d columns per core; KC: number of 128-pole chunks;
    T: timesteps; XG: pole-chunks per x-load group.
    """
    NP_ = KC * 128          # poles
    KTOT = 1 + 4 * NP_      # rows of x
    TH = T // 2             # 512 per parity
    DH = DSH // 128         # d-half count per core

    nc = bacc.Bacc("TRN2", target_bir_lowering=False, debug=False)

    rr_d = nc.dram_tensor("rr", [NP_], F32, kind="ExternalInput")
    th_d = nc.dram_tensor("theta", [NP_], F32, kind="ExternalInput")
    x_d = nc.dram_tensor("x", [B, KTOT, DSH], F32, kind="ExternalInput")
    out_d = nc.dram_tensor("out", [B, DSH, T], F32, kind="ExternalOutput")

    with tile.TileContext(nc) as tc, ExitStack() as ctx:
        const = ctx.enter_context(tc.tile_pool(name="const", bufs=1))
        pwp = ctx.enter_context(tc.tile_pool(name="pwp", bufs=2))
        rp = ctx.enter_context(tc.tile_pool(name="rp", bufs=2))
        csp = ctx.enter_context(tc.tile_pool(name="csp", bufs=1))
        tiny = ctx.enter_context(tc.tile_pool(name="tiny", bufs=4))
        xp = ctx.enter_context(tc.tile_pool(name="xp", bufs=2))
        uvp = ctx.enter_context(tc.tile_pool(name="uv", bufs=2))
        outp = ctx.enter_context(tc.tile_pool(name="outp", bufs=2))
        psp = ctx.enter_context(
            tc.tile_pool(name="ps", bufs=2, space=bass.MemorySpace.PSUM)
        )

        # ---- setup ----------------------------------------------------
        rr_t = const.tile([128, KC], F32, tag="rr")
        th_t = const.tile([128, KC], F32, tag="th")
        nc.sync.dma_start(rr_t[:], rr_d[:].rearrange("(kc p) -> p kc", p=128))
        nc.sync.dma_start(th_t[:], th_d[:].rearrange("(kc p) -> p kc", p=128))

        iota_i = rp.tile([128, TH], I32, tag="q", name="iota_i")
        nc.gpsimd.iota(iota_i[:], pattern=[[1, TH]], base=0,
                       channel_multiplier=0)
        iota_f = const.tile([128, TH], F32, tag="iotaf")
        nc.vector.tensor_copy(iota_f[:], iota_i[:])

        # ones-column bias: x[b,0,d] / sqrt(T), per (dh, b)
        x0s = const.tile([128, DH * B], F32, tag="x0s")
        for b in range(B):
            nc.sync.dma_start(
                x0s[:, b * DH:(b + 1) * DH],
                x_d[b, 0, :].rearrange("(dh p) -> p dh", p=128),
            )
        x0sc = const.tile([128, DH * B], F32, tag="x0sc")
        nc.vector.tensor_scalar_mul(x0sc[:], x0s[:], 1.0 / float(np.sqrt(T)))

        # Closed-form column norms.  With R = r^2, z = R e^{2i th},
        # S0 = sum_t R^t and C = sum_t z^t (t = 0..T-1, geometric series):
        #   G_A^2 = (S0 + Re C)/2,   G_B^2 = (S0 - Re C)/2.
        # Computed cancellation-free: 1-R = (1-r)(1+r), Re(1-z) =
        # (1-R) + 2R sin^2(th), and the difference S0 - Re C is evaluated
        # as Re[N/D] with N = (R-z) - R^T(1-z) + z^T(1-R), D = (1-R)(1-z),
        # whose leading term 2R sin^2(th) is a positive product (no
        # subtractive cancellation for near-zero-norm columns).
        sinth = const.tile([128, KC], F32, tag="sinth")
        nc.scalar.activation(sinth[:], th_t[:], AF.Sin, bias=0.0, scale=1.0)
        hpi = const.tile([128, 1], F32, tag="hpi")
        nc.vector.memset(hpi[:], PI / 2)
        bigp = const.tile([128, 1], F32, tag="bigp")
        nc.vector.memset(bigp[:], RND_BIG)
        bigm = const.tile([128, 1], F32, tag="bigm")
        nc.vector.memset(bigm[:], -RND_BIG)
        costh = const.tile([128, KC], F32, tag="costh")
        nc.scalar.activation(costh[:], th_t[:], AF.Sin, bias=hpi[:],
                             scale=1.0)
        rs = const.tile([128, KC], F32, tag="rs")
        nc.vector.tensor_mul(rs[:], sinth[:], rr_t[:])
        maskB = const.tile([128, KC], F32, tag="maskB")
        nc.vector.tensor_scalar(maskB[:], rs[:], 0.0, None, op0=OP.is_gt)
        # theta/pi, so that q = j*(th/pi) = (2j)*th/(2pi) counts turns
        thp = const.tile([128, KC], F32, tag="thp")
        nc.vector.tensor_scalar_mul(thp[:], th_t[:], 1.0 / PI)

        # (r^2)^(2^j) per pole (doubling build of r^(2j), parity-major pw)
        r2j = const.tile([128, 10, KC], F32, tag="r2j")
        nc.vector.tensor_mul(r2j[:, 0], rr_t[:], rr_t[:])
        for j in range(1, 10):
            nc.vector.tensor_mul(r2j[:, j], r2j[:, j - 1], r2j[:, j - 1])

        cfp = ctx.enter_context(tc.tile_pool(name="cfp", bufs=1))

        def cf(name):
            return cfp.tile([128, KC], F32, tag=name, name=name)

        R_ = r2j[:, 0]
        rt = cf("rt")                      # R^T = r^2048
        nc.vector.tensor_mul(rt[:], r2j[:, 9], r2j[:, 9])
        omr = cf("omr")
        nc.vector.tensor_scalar(omr[:], rr_t[:], -1.0, 1.0,
                                op0=OP.mult, op1=OP.add)
        opr = cf("opr")
        nc.vector.tensor_scalar(opr[:], rr_t[:], 1.0, None, op0=OP.add)
        omR = cf("omR")
        nc.vector.tensor_mul(omR[:], omr[:], opr[:])
        ssq2 = cf("ssq2")                  # 2 sin^2(th)
        nc.vector.scalar_tensor_tensor(ssq2[:], sinth[:], 2.0, sinth[:],
                                       op0=OP.mult, op1=OP.mult)
        s2t = cf("s2t")                    # sin(2 th)
        nc.vector.scalar_tensor_tensor(s2t[:], sinth[:], 2.0, costh[:],
                                       op0=OP.mult, op1=OP.mult)
        zim = cf("zim")                    # Im z = R sin(2 th)
        nc.vector.tensor_mul(zim[:], R_, s2t[:])
        rmz = cf("rmz")                    # Re(R - z) = 2 R sin^2(th)
        nc.vector.tensor_mul(rmz[:], R_, ssq2[:])
        a1r = cf("a1r")                    # Re(1 - z)
        nc.vector.tensor_add(a1r[:], omR[:], rmz[:])
        # angle 2*T*th mod 2pi via the turn trick (T = 1024)
        qq = cf("qq")
        nc.vector.tensor_scalar(qq[:], thp[:], 1024.0, None, op0=OP.mult)
        qqr = cf("qqr")
        nc.vector.tensor_scalar(qqr[:], qq[:], RND_BIG, -RND_BIG,
                                op0=OP.add, op1=OP.add)
        dd = cf("dd")
        nc.vector.scalar_tensor_tensor(dd[:], qqr[:], -1.0, qq[:],
                                       op0=OP.mult, op1=OP.add)
        ddc = cf("ddc")
        nc.vector.tensor_scalar(ddc[:], dd[:], 0.5, -0.5,
                                op0=OP.min, op1=OP.max)
        adt = cf("adt")
        nc.vector.tensor_scalar(adt[:].bitcast(I32), dd[:].bitcast(I32),
                                0x7FFFFFFF, None, op0=OP.bitwise_and)
        sT = cf("sT")
        nc.scalar.activation(sT[:], ddc[:], AF.Sin, bias=0.0, scale=TWO_PI)
        cT = cf("cT")
        nc.scalar.activation(cT[:], adt[:], AF.Sin, bias=hpi[:],
                             scale=-TWO_PI)
        zTr = cf("zTr")
        nc.vector.tensor_mul(zTr[:], rt[:], cT[:])
        zTi = cf("zTi")
        nc.vector.tensor_mul(zTi[:], rt[:], sT[:])
        omrt = cf("omrt")                  # 1 - R^T
        nc.vector.tensor_scalar(omrt[:], rt[:], -1.0, 1.0,
                                op0=OP.mult, op1=OP.add)
        rrec = cf("rrec")
        nc.vector.reciprocal(rrec[:], omR[:])
        s0_ = cf("s0_")                    # S0 = (1-R^T)/(1-R)
        nc.vector.tensor_mul(s0_[:], omrt[:], rrec[:])
        # Re C = ((1-zTr) a1r + zTi zim) / (a1r^2 + zim^2)
        xx = cf("xx")
        nc.vector.tensor_scalar(xx[:], zTr[:], -1.0, 1.0,
                                op0=OP.mult, op1=OP.add)
        n1 = cf("n1")
        nc.vector.tensor_mul(n1[:], xx[:], a1r[:])
        n2 = cf("n2")
        nc.vector.tensor_mul(n2[:], zTi[:], zim[:])
        num = cf("num")
        nc.vector.tensor_add(num[:], n1[:], n2[:])
        dn1 = cf("dn1")
        nc.vector.tensor_mul(dn1[:], a1r[:], a1r[:])
        dn2 = cf("dn2")
        nc.vector.tensor_mul(dn2[:], zim[:], zim[:])
        den = cf("den")
        nc.vector.tensor_add(den[:], dn1[:], dn2[:])
        rden = cf("rden")
        nc.vector.reciprocal(rden[:], den[:])
        reC = cf("reC")
        nc.vector.tensor_mul(reC[:], num[:], rden[:])
        # pack G^2: [:,0,:] = A block, [:,1,:] = B block
        g2t = const.tile([128, 2, KC], F32, tag="g2t")
        nc.vector.tensor_add(g2t[:, 0], s0_[:], reC[:])
        nc.vector.tensor_scalar_mul(g2t[:, 0], g2t[:, 0], 0.5)
        # G_B^2 = Re[N/D]/2
        nr1 = cf("nr1")
        nc.vector.tensor_mul(nr1[:], rt[:], a1r[:])
        nr2 = cf("nr2")
        nc.vector.tensor_mul(nr2[:], zTr[:], omR[:])
        nre = cf("nre")
        nc.vector.tensor_sub(nre[:], rmz[:], nr1[:])
        nc.vector.tensor_add(nre[:], nre[:], nr2[:])
        ni1 = cf("ni1")
        nc.vector.tensor_mul(ni1[:], zim[:], omrt[:])
        ni2 = cf("ni2")
        nc.vector.tensor_mul(ni2[:], zTi[:], omR[:])
        nim = cf("nim")
        nc.vector.tensor_sub(nim[:], ni2[:], ni1[:])
        dre = cf("dre")
        nc.vector.tensor_mul(dre[:], omR[:], a1r[:])
        dimp = cf("dimp")                  # -Im D
        nc.vector.tensor_mul(dimp[:], omR[:], zim[:])
        m1_ = cf("m1_")
        nc.vector.tensor_mul(m1_[:], nre[:], dre[:])
        m2_ = cf("m2_")
        nc.vector.tensor_mul(m2_[:], nim[:], dimp[:])
        mnum = cf("mnum")
        nc.vector.tensor_sub(mnum[:], m1_[:], m2_[:])
        e1_ = cf("e1_")
        nc.vector.tensor_mul(e1_[:], dre[:], dre[:])
        e2_ = cf("e2_")
        nc.vector.tensor_mul(e2_[:], dimp[:], dimp[:])
        eden = cf("eden")
        nc.vector.tensor_add(eden[:], e1_[:], e2_[:])
        rede = cf("rede")
        nc.vector.reciprocal(rede[:], eden[:])
        nc.vector.tensor_mul(g2t[:, 1], mnum[:], rede[:])
        nc.vector.tensor_scalar_mul(g2t[:, 1], g2t[:, 1], 0.5)
        # invg = 1/sqrt(max(g2, 1e-30)) via Newton, one pass over both blocks
        gcl = const.tile([128, 2, KC], F32, tag="gcl")
        nc.vector.tensor_scalar_max(gcl[:], g2t[:], 1e-30)
        y0i = const.tile([128, 2, KC], I32, tag="y0i")
        nc.vector.tensor_scalar(y0i[:], gcl[:].bitcast(I32), 1, None,
                                op0=OP.arith_shift_right)
        invgt = const.tile([128, 2, KC], F32, tag="invgt")
        y_t = invgt
        nc.vector.tensor_scalar(y_t[:].bitcast(I32), y0i[:], -1,
                                RSQRT_MAGIC, op0=OP.mult, op1=OP.add)
        yy = const.tile([128, 2, KC], F32, tag="yy")
        ff = const.tile([128, 2, KC], F32, tag="ff")
        for it in range(NEWTON + 1):
            nc.vector.tensor_mul(yy[:], y_t[:], y_t[:])
            nc.vector.tensor_mul(yy[:], yy[:], gcl[:])
            nc.vector.tensor_scalar(ff[:], yy[:], -0.5, 1.5,
                                    op0=OP.mult, op1=OP.add)
            nc.vector.tensor_mul(y_t[:], y_t[:], ff[:])
        invgbm = const.tile([128, KC], F32, tag="invgbm")
        nc.vector.tensor_mul(invgbm[:], invgt[:, 1], maskB[:])

        # ---- dictionary -----------------------------------------------
        adict = const.tile([128, KC, 2, TH], F16, tag="adict")
        bdict = const.tile([128, KC, 2, TH], F16, tag="bdict")

        # Software-pipelined build: emit S0(k), S1(k-1), S2(k-2)
        # interleaved so every engine always has ready work (engines issue
        # in program order with a 4-deep wait queue; a naive per-kc chain
        # crosses engines ~10 times and serializes).
        st = {}

        def s0(kc):
            # pw[par, j] = r^(2j+par) by doubling r^2; q = j*th/pi (turns)
            pw = pwp.tile([128, 2, TH], F32, tag="pw", name="pw")
            nc.vector.memset(pw[:, 0, 0:1], 1.0)
            nc.vector.tensor_copy(pw[:, 0, 1:2], r2j[:, 0, kc:kc + 1])
            n = 2
            j = 1
            while n < TH:
                nc.vector.tensor_scalar(pw[:, 0, n:2 * n], pw[:, 0, 0:n],
                                        r2j[:, j, kc:kc + 1], None,
                                        op0=OP.mult)
                n *= 2
                j += 1
            nc.vector.tensor_scalar(pw[:, 1, :], pw[:, 0, :],
                                    rr_t[:, kc:kc + 1], None, op0=OP.mult)
            q = rp.tile([128, TH], F32, tag="q", name="q")
            nc.scalar.activation(q[:], iota_f[:], AF.Identity, bias=0.0,
                                 scale=thp[:, kc:kc + 1])
            qh = rp.tile([128, TH], F32, tag="qh", name="qh")
            nc.scalar.activation(qh[:], q[:], AF.Identity, bias=bigp[:],
                                 scale=1.0)
            qr = rp.tile([128, TH], F32, tag="qr", name="qr")
            nc.scalar.activation(qr[:], qh[:], AF.Identity, bias=bigm[:],
                                 scale=1.0)
            st[kc] = {"pw": pw, "q": q, "qr": qr}

        def s1(kc):
            z = st[kc]
            d_t = rp.tile([128, TH], F32, tag="d", name="d_t")
            nc.vector.scalar_tensor_tensor(d_t[:], z["qr"][:], -1.0,
                                           z["q"][:], op0=OP.mult,
                                           op1=OP.add)
            dc = rp.tile([128, TH], F32, tag="dc", name="dc")
            nc.vector.tensor_scalar(dc[:], d_t[:], 0.5, -0.5,
                                    op0=OP.min, op1=OP.max)
            # cos(2*pi*d) = sin(pi/2 - 2*pi*|d|), |d| <= 1/2 stays in range
            absd = rp.tile([128, TH], F32, tag="d", name="absd")
            nc.vector.tensor_scalar(absd[:].bitcast(I32),
                                    d_t[:].bitcast(I32), 0x7FFFFFFF, None,
                                    op0=OP.bitwise_and)
            c_t = csp.tile([128, 2, TH], F32, tag="c", name="c_t")
            s_t = csp.tile([128, 2, TH], F32, tag="s", name="s_t")
            nc.scalar.activation(s_t[:, 0], dc[:], AF.Sin, bias=0.0,
                                 scale=TWO_PI)
            nc.scalar.activation(c_t[:, 0], absd[:], AF.Sin, bias=hpi[:],
                                 scale=-TWO_PI)
            nc.vector.memset(s_t[:, 0, 0:1], 0.0)
            nc.vector.memset(c_t[:, 0, 0:1], 1.0)
            z.update(c=c_t, s=s_t)

        def s2(kc):
            z = st[kc]
            c_t, s_t, pw = z["c"], z["s"], z["pw"]
            ta = rp.tile([128, TH], F32, tag="q", name="ta")
            nc.vector.tensor_scalar(ta[:], s_t[:, 0], sinth[:, kc:kc + 1],
                                    None, op0=OP.mult)
            nc.vector.scalar_tensor_tensor(c_t[:, 1], c_t[:, 0],
                                           costh[:, kc:kc + 1], ta[:],
                                           op0=OP.mult, op1=OP.subtract)
            tb = rp.tile([128, TH], F32, tag="qr", name="tb")
            nc.vector.tensor_scalar(tb[:], c_t[:, 0], sinth[:, kc:kc + 1],
                                    None, op0=OP.mult)
            nc.vector.scalar_tensor_tensor(s_t[:, 1], s_t[:, 0],
                                           costh[:, kc:kc + 1], tb[:],
                                           op0=OP.mult, op1=OP.add)
            # normalized fp16 dictionary, written directly
            nc.vector.scalar_tensor_tensor(adict[:, kc], c_t[:],
                                           invgt[:, 0, kc:kc + 1], pw[:],
                                           op0=OP.mult, op1=OP.mult)
            nc.vector.scalar_tensor_tensor(bdict[:, kc], s_t[:],
                                           invgbm[:, kc:kc + 1], pw[:],
                                           op0=OP.mult, op1=OP.mult)
            del st[kc]

        # ---- GEMM --------------------------------------------------
        # b0/b1 are interleaved with the dictionary build (their matmuls
        # consume dict chunks as they land; combines run on Pool which is
        # idle during the dict phase). b2/b3 run after, combines on DVE
        # (idle once the dict is done).
        ps = {}

        def gemm_open(b):
            ps[b] = ([psp.tile([128, TH], F32, tag=f"pe{dh}",
                               name=f"pse{dh}") for dh in range(DH)],
                     [psp.tile([128, TH], F32, tag=f"po{dh}",
                               name=f"pso{dh}") for dh in range(DH)])

        def gemm_load(b, g, eng):
            xt = xp.tile([128, 4, XG, DSH], F32, tag="x", name="xt")
            for i in range(XG):
                nc.sync.dma_start(
                    xt[:, :, i],
                    x_d[b, 1:, :].rearrange(
                        "(blk kc p) d -> p blk kc d", blk=4,
                        kc=KC, p=128)[:, :, g * XG + i],
                )
            u_t = uvp.tile([128, XG, DSH], F16, tag="u", name="u_t")
            v_t = uvp.tile([128, XG, DSH], F16, tag="v", name="v_t")
            w_t = uvp.tile([128, XG, DSH], F16, tag="w", name="w_t")
            z_t = uvp.tile([128, XG, DSH], F16, tag="z", name="z_t")
            e0 = nc.gpsimd if eng == "pool" else nc.vector
            e1 = nc.gpsimd if eng == "pool" else nc.vector
            e0.tensor_add(u_t[:], xt[:, 0], xt[:, 1])
            e0.tensor_sub(v_t[:], xt[:, 0], xt[:, 1])
            e1.tensor_add(w_t[:], xt[:, 2], xt[:, 3])
            e1.tensor_sub(z_t[:], xt[:, 2], xt[:, 3])
            return u_t, v_t, w_t, z_t

        def gemm_kc(b, kc, uvwz):
            u_t, v_t, w_t, z_t = uvwz
            ps_e, ps_o = ps[b]
            i = kc % XG
            first = kc == 0
            last = kc == KC - 1
            for dh in range(DH):
                dsl = slice(dh * 128, (dh + 1) * 128)
                nc.tensor.matmul(ps_e[dh][:], u_t[:, i, dsl],
                                 adict[:, kc, 0, :], start=first,
                                 stop=False)
                nc.tensor.matmul(ps_o[dh][:], v_t[:, i, dsl],
                                 adict[:, kc, 1, :], start=first,
                                 stop=False)
                nc.tensor.matmul(ps_e[dh][:], w_t[:, i, dsl],
                                 bdict[:, kc, 0, :], start=False, stop=last)
                nc.tensor.matmul(ps_o[dh][:], z_t[:, i, dsl],
                                 bdict[:, kc, 1, :], start=False, stop=last)

        def gemm_close(b):
            ps_e, ps_o = ps.pop(b)
            for dh in range(DH):
                col = b * DH + dh
                ob_e = outp.tile([128, TH], F32, tag="ob", name="ob_e")
                ob_o = outp.tile([128, TH], F32, tag="ob", name="ob_o")
                nc.scalar.activation(ob_e[:], ps_e[dh][:], AF.Identity,
                                     bias=x0sc[:, col:col + 1], scale=1.0)
                nc.scalar.activation(ob_o[:], ps_o[dh][:], AF.Identity,
                                     bias=x0sc[:, col:col + 1], scale=1.0)
                rows = slice(dh * 128, (dh + 1) * 128)
                nc.sync.dma_start(out_d[b, rows, 0:TH], ob_e[:])
                nc.sync.dma_start(out_d[b, rows, TH:T], ob_o[:])

        # phase 1: dict stages interleaved with b0/b1 consumption
        p1 = [b for b in (0, 1) if b < B]
        gemm_open(0)
        if 1 in p1:
            gemm_open(1)
        uvwz01 = {}
        for k in range(KC + 2):
            if k < KC:
                s0(k)
            if 1 <= k < KC + 1:
                s1(k - 1)
            if k >= 2:
                kc = k - 2
                s2(kc)
                if kc % XG == 0:
                    g = kc // XG
                    for b in p1:
                        uvwz01[b] = gemm_load(b, g, "pool")
                for b in p1:
                    gemm_kc(b, kc, uvwz01[b])
        for b in p1:
            gemm_close(b)

        # phase 2: remaining batches at full speed, combines on DVE
        for b in range(2, B):
            gemm_open(b)
            for g in range(KC // XG):
                uvwz = gemm_load(b, g, "dve")
                for i in range(XG):
                    gemm_kc(b, g * XG + i, uvwz)
            gemm_close(b)
    nc.compile()
    return nc


_NC_CACHE = {}


def _get_nc(key, **kw):
    if key not in _NC_CACHE:
        _NC_CACHE[key] = build_kernel_nc(**kw)
    return _NC_CACHE[key]


def assemble_output(core_outs, B=4, T=1024, D=2048):
    """core_outs: list of [B, DSH, T] arrays (parity-major t) -> [B, T, D]."""
    dsh = D // len(core_outs)
    th = T // 2
    out = np.empty((B, T, D), dtype=np.float32)
    for c, oc in enumerate(core_outs):
        dsl = slice(c * dsh, (c + 1) * dsh)
        out[:, 0::2, dsl] = np.swapaxes(oc[:, :, :th], 1, 2)
        out[:, 1::2, dsl] = np.swapaxes(oc[:, :, th:], 1, 2)
    return out


def kernel(rr, theta, x, trace=False, trace_kwargs=None):
    rr = np.ascontiguousarray(np.asarray(rr, dtype=np.float32))
    theta = np.ascontiguousarray(np.asarray(theta, dtype=np.float32))
    x = np.asarray(x, dtype=np.float32)
    B, KTOT, D = x.shape
    dsh = D // N_CORES
    nc = _get_nc("full")
    in_maps = []
    for c in range(N_CORES):
        in_maps.append({
            "rr": rr,
            "theta": theta,
            "x": np.ascontiguousarray(x[:, :, c * dsh:(c + 1) * dsh]),
        })
    kw = {}
    if trace:
        kw = {"trace": True, "trace_kwargs": trace_kwargs or {}}
    res = bass_utils.run_bass_kernel_spmd(nc, in_maps,
                                          core_ids=list(range(N_CORES)), **kw)
    out = assemble_output([res.results[c]["out"] for c in range(N_CORES)],
                          B=B, T=1024, D=D)
    if trace:
        return out, res
    return out
